# revision 10
# baseline (speedup 1.0000x reference)
"""Bass/Tile kernel for nn_BinaryClassifierChain on 8 trn2 cores (v6).

Math (per reference.py):
  wc   = softmax(word_class_features, axis=0)            # over batch dim
  base = concat([features, wc], -1)                      # [B, W, 1088]
  L    = base @ W[:, :1088].T + b                        # [B, W, 32]
  chain: p_i = sigmoid(L_i + sum_{j<i} Wbin[i, j] p_j)   # Wbin = W[:, 1088:]

Sharding: pure data-parallel over the words dim (1024 = 8 x 128); the
batch-softmax stays intact per shard.

v6 vs v5 (v5 trace: 55us serial-chain tail after the matmul pipeline;
stores fixed but chains latency-bound at ~1.3us/bin-step):
  - four chain chunks of 16 batches: A (0-15) and B (16-31) run the
    mul+reduce+sigmoid form fully hidden inside the pipeline (groups
    4-11 / 8-15); C (32-47) and D (48-63) use a low-latency FMA form:
    logits accumulate in a f32 tile ZF via rank-1 updates so the
    critical path per bin is one DVE FMA + one ACT sigmoid (~0.7us),
    with the lagged rank-1 mul offloaded to the idle GpSimd engine.
  - load order: identity first, then group feature halves; wc halves
    early on both rings; weight tables behind them.
  - x2 prefetch depth 6 groups.
  - stores issued per chunk the moment its chain finishes.
"""

import sys

sys.path.insert(0, "/opt/trn_rl_repo")

import numpy as np
import orjson
import ml_dtypes

import concourse.bass as bass
import concourse.mybir as mybir
import concourse.tile as tile
from concourse import masks
from concourse.bass_utils import run_bass_kernel_spmd

F32 = mybir.dt.float32
F32R = mybir.dt.float32r
BF16 = mybir.dt.bfloat16
AF = mybir.ActivationFunctionType
ALU = mybir.AluOpType
AX = mybir.AxisListType

B = 64          # batch
NWALL = 1024    # total words
NCORES = 8
NW = NWALL // NCORES  # 128 words per core
D = 1024        # embed dim
C = 64          # word classes
NB = 32         # bin features
DIN = D + C + NB  # 1120
GRP = 4         # batches per matmul group (4 * 128 words = 512 tokens)
NGRP = B // GRP
NCH = 16        # batches per chain chunk

# lagged rank-1 updates of the FMA chains go to gpsimd (idle engine);
# flip to False if the Pool tensor_tensor path fails to compile.
R_ON_GPSIMD = True


def _split_multiwait_json(raw: bytes) -> bytes:
    """walrus in this container only accepts 1 sync-wait per most
    instructions; Tile's final drain (and some others) carry several.
    Move extras onto preceding EventSemaphore carriers (2 waits each) on
    the same engine."""
    bir = orjson.loads(raw)
    for fn in bir["functions"]:
        for blk in fn["blocks"]:
            out = []
            for ins in blk["instructions"]:
                si = ins.get("sync_info")
                waits = (si or {}).get("on_wait") or []
                if len(waits) > 1:
                    extra = waits[:-1]
                    for k in range(0, len(extra), 2):
                        out.append(
                            {
                                "debug": ins.get("debug", 0),
                                "engine": ins["engine"],
                                "ins": [],
                                "outs": [],
                                "name": f"{ins['name']}_sw{k}",
                                "opcode": "EventSemaphore",
                                "sync_info": {
                                    "on_update": [],
                                    "on_wait": extra[k : k + 2],
                                },
                            }
                        )
                    si["on_wait"] = [waits[-1]]
                out.append(ins)
            blk["instructions"] = out
    return orjson.dumps(bir)


def build_program():
    nc = bass.Bass("TRN2", target_bir_lowering=False, debug=False)

    feat = nc.dram_tensor("feat", [B, NW, D], F32R, kind="ExternalInput")
    wc = nc.dram_tensor("wc", [B, NW, C], F32, kind="ExternalInput")
    wtrd = nc.dram_tensor("wtr", [128, 9, NB], BF16, kind="ExternalInput")
    vrd = nc.dram_tensor("vrows", [128, NB, NB], BF16, kind="ExternalInput")
    bt = nc.dram_tensor("b", [NB, 128], F32, kind="ExternalInput")
    identd = nc.dram_tensor("ident", [128, 128], F32R, kind="ExternalInput")
    # FMA-chain tables: first sub-diagonal of Wbin, and its columns
    # (rows >= j+2) for the lagged rank-1 updates.
    tsubd = nc.dram_tensor("tsub", [128, NB], F32, kind="ExternalInput")
    vcold = nc.dram_tensor("vcolf", [128, NB, NB], F32, kind="ExternalInput")
    # out stays word-major ([w, b, i], matching Z's layout) so stores are
    # contiguous runs at line rate; the host transposes axes 0/1 after.
    out = nc.dram_tensor("out", [NW, B, NB], BF16, kind="ExternalOutput")

    with tile.TileContext(nc) as tc:
        with (
            tc.tile_pool(name="const", bufs=1) as constp,
            tc.tile_pool(name="x2", bufs=6) as x2p,
            tc.tile_pool(name="xt", bufs=2) as xtp,
            tc.tile_pool(name="blt", bufs=2) as bltp,
            tc.tile_pool(name="tp", bufs=2, space="PSUM") as tpp,
            tc.tile_pool(name="wcps", bufs=1, space="PSUM") as wcpsp,
            tc.tile_pool(name="mmps", bufs=2, space="PSUM") as mmpsp,
            tc.tile_pool(name="petps", bufs=1, space="PSUM") as petpsp,
        ):
            # f32r identity from host (gpsimd memset can't touch f32r)
            identr = constp.tile([128, 128], F32R)
            nc.scalar.dma_start(identr[:], identd.ap())

            # wc halves early on both rings (softmax gates group 0's
            # final matmul); feature group halves follow.
            wcs = constp.tile([128, B, C], F32)
            wc_r = wc.ap().rearrange("b p c -> p b c")
            nc.sync.dma_start(wcs[0:64], wc_r[0:64])

            ident = constp.tile([128, 128], BF16)
            masks.make_identity(nc, ident[:])
            identf = constp.tile([128, 128], F32)
            masks.make_identity(nc, identf[:])

            # group-0/1 feature halves ahead of the weight tables
            x2_tiles = []
            for g in range(2):
                b0 = g * GRP
                x2 = x2p.tile([128, GRP, D], F32R, tag="x2")
                fr = feat.ap()[b0 : b0 + GRP, :, :].rearrange("b p d -> p b d")
                nc.sync.dma_start(x2[:, :, 0 : D // 2], fr[:, :, 0 : D // 2])
                nc.scalar.dma_start(x2[:, :, D // 2 : D], fr[:, :, D // 2 : D])
                x2_tiles.append(x2)
            nc.scalar.dma_start(wcs[64:128], wc_r[64:128])

            b_sb = constp.tile([NB, 128], F32)
            nc.scalar.dma_start(b_sb[:], bt.ap())
            wtr = constp.tile([128, 9, NB], BF16)
            nc.scalar.dma_start(wtr[:], wtrd.ap())
            vr = constp.tile([128, NB, NB], BF16)
            nc.scalar.dma_start(vr[:], vrd.ap())
            tsub = constp.tile([128, NB], F32)
            nc.scalar.dma_start(tsub[:], tsubd.ap())
            vcolf = constp.tile([128, NB, NB], F32)
            nc.scalar.dma_start(vcolf[:], vcold.ap())

            wcn = constp.tile([128, B, C], BF16)
            # token-major chain state: [words, batch, bins]; for batches
            # 0-31 slot i holds L_i until the sigmoid replaces it with
            # p_i; for batches 32-63 it only ever holds probabilities
            # (logits live in ZF).
            Z = constp.tile([128, B, NB], BF16)
            # f32 logit accumulators for the FMA chains (batches 32-63)
            ZF = constp.tile([128, 2 * NCH, NB], F32)
            tmpA = constp.tile([128, NCH, NB + 1], BF16)
            zcA = constp.tile([128, NCH], F32)
            tmpB = constp.tile([128, NCH, NB + 1], BF16)
            zcB = constp.tile([128, NCH], F32)
            rtC = constp.tile([128, NCH, NB], F32)
            rtD = constp.tile([128, NCH, NB], F32)

            # ---------------- softmax over batch ----------------
            with tc.tile_pool(name="soft", bufs=1) as softp:
                ex = softp.tile([128, B, C], F32)
                nc.scalar.activation(ex[:], wcs[:], AF.Exp)
                acc = softp.tile([128, B // 2, C], F32)
                nc.vector.tensor_add(
                    acc[:], ex[:, 0 : B // 2, :], ex[:, B // 2 : B, :]
                )
                h = B // 4
                while h >= 1:
                    nc.vector.tensor_add(
                        acc[:, 0:h, :], acc[:, 0:h, :], acc[:, h : 2 * h, :]
                    )
                    h //= 2
                rec = softp.tile([128, C], F32)
                nc.vector.reciprocal(rec[:], acc[:, 0, :])
                nc.vector.tensor_mul(
                    wcn[:],
                    ex[:],
                    rec[:].unsqueeze(1).broadcast_to([128, B, C]),
                )

            # ---------------- chain helpers ----------------
            def chain_bin(i, bs, tmp, zc):
                nbt = bs.stop - bs.start
                if i == 0:
                    nc.scalar.activation(Z[:, bs, 0], Z[:, bs, 0], AF.Sigmoid)
                    return
                nc.vector.tensor_mul(
                    tmp[:, :, 0 : i + 1],
                    Z[:, bs, 0 : i + 1],
                    vr[:, i, 0 : i + 1]
                    .unsqueeze(1)
                    .broadcast_to([128, nbt, i + 1]),
                )
                nc.vector.reduce_sum(zc[:, :], tmp[:, :, 0 : i + 1], axis=AX.X)
                nc.scalar.activation(Z[:, bs, i], zc[:, :], AF.Sigmoid)

            def fma_bin(i, c0, rt):
                """FMA-form chain step for ZF chunk [c0, c0+NCH):
                sigmoid of the accumulated logit, then push p_i into the
                next slot (critical, DVE) and all later slots (lagged
                rank-1, gpsimd mul + DVE add)."""
                cs = slice(c0, c0 + NCH)
                zs = slice(2 * NCH + c0, 2 * NCH + c0 + NCH)
                nc.scalar.activation(Z[:, zs, i], ZF[:, cs, i], AF.Sigmoid)
                if i + 1 < NB:
                    nc.vector.scalar_tensor_tensor(
                        ZF[:, cs, i + 1],
                        Z[:, zs, i],
                        tsub[:, i : i + 1],
                        ZF[:, cs, i + 1],
                        op0=ALU.mult,
                        op1=ALU.add,
                    )
                if i + 2 < NB:
                    w = NB - (i + 2)
                    mul_eng = nc.gpsimd if R_ON_GPSIMD else nc.vector
                    mul_eng.tensor_mul(
                        rt[:, :, 0:w],
                        Z[:, zs, i : i + 1].broadcast_to([128, NCH, w]),
                        vcolf[:, i, i + 2 : NB]
                        .unsqueeze(1)
                        .broadcast_to([128, NCH, w]),
                    )
                    nc.vector.tensor_add(
                        ZF[:, cs, i + 2 : NB], ZF[:, cs, i + 2 : NB], rt[:, :, 0:w]
                    )

            def store_chunk(b0, b1):
                nc.sync.dma_start(out.ap()[:, b0:b1, :], Z[:, b0:b1, :])

            bsA = slice(0, NCH)
            bsB = slice(NCH, 2 * NCH)

            # per-group chain-step schedule (emitted at hook points):
            # A bins 0-31 over groups 4-11, B over 8-15 (mul+reduce,
            # latency fully hidden); C bins 0-15 over groups 12-15 (FMA).
            sched = {g: [] for g in range(NGRP)}
            for i in range(NB):
                sched[4 + i // 4].append(
                    lambda i=i: chain_bin(i, bsA, tmpA, zcA)
                )
            for i in range(NB):
                sched[8 + i // 4].append(
                    lambda i=i: chain_bin(i, bsB, tmpB, zcB)
                )
            for i in range(NCH):
                sched[12 + i // 4].append(lambda i=i: fma_bin(i, 0, rtC))
            # interleave so consecutive emissions never belong to the
            # same serial chain
            for g in range(NGRP):
                items = sched[g]
                if len(items) == 8:
                    sched[g] = [
                        items[j // 2] if j % 2 == 0 else items[4 + j // 2]
                        for j in range(8)
                    ]

            # ---------------- main matmul pipeline ----------------
            for g in range(NGRP):
                b0 = g * GRP
                todo = list(sched[g])
                n_hooks = 6
                per = (len(todo) + n_hooks - 1) // n_hooks if todo else 0

                def emit():
                    for _ in range(per):
                        if todo:
                            todo.pop(0)()

                if g < 2:
                    x2 = x2_tiles[g]
                else:
                    x2 = x2p.tile([128, GRP, D], F32R, tag="x2")
                    fr = feat.ap()[b0 : b0 + GRP, :, :].rearrange(
                        "b p d -> p b d"
                    )
                    nc.sync.dma_start(x2[:, :, 0 : D // 2], fr[:, :, 0 : D // 2])
                    nc.scalar.dma_start(x2[:, :, D // 2 : D], fr[:, :, D // 2 : D])
                xts = xtp.tile([128, 9, 512], BF16, tag="xt")
                for kh in range(4):
                    pt = tpp.tile([128, 2, 512], F32R, tag="xtps")
                    for kk in range(2):
                        k = kh * 2 + kk
                        for bi in range(GRP):
                            nc.tensor.transpose(
                                pt[:, kk, bi * 128 : (bi + 1) * 128],
                                x2[:, bi, k * 128 : (k + 1) * 128],
                                identr[:],
                            )
                    # cast-evacuation f32 psum -> bf16 SBUF, split ACT/DVE
                    if kh % 2 == 0:
                        nc.scalar.copy(xts[:, kh * 2 : kh * 2 + 2, :], pt[:])
                    else:
                        nc.vector.tensor_copy(xts[:, kh * 2 : kh * 2 + 2, :], pt[:])
                    if kh < 2:
                        emit()

                # softmaxed wc as 9th k-chunk: transpose on chip
                wps = wcpsp.tile([64, 512], BF16, tag="wct")
                for bi in range(GRP):
                    nc.tensor.transpose(
                        wps[:, bi * 128 : (bi + 1) * 128],
                        wcn[:, b0 + bi, :],
                        ident[:],
                    )
                nc.scalar.copy(xts[0:64, 8, :], wps[:])
                emit()
                emit()

                ps = mmpsp.tile([NB, 512], F32, tag="mm")
                for k in range(8):
                    nc.tensor.matmul(
                        ps[:], wtr[:, k, :], xts[:, k, :],
                        start=(k == 0), stop=False,
                    )
                nc.tensor.matmul(
                    ps[:], wtr[0:64, 8, :], xts[0:64, 8, :],
                    start=False, stop=True,
                )
                blt = bltp.tile([NB, 512], F32, tag="blt")
                nc.scalar.activation(
                    blt[:], ps[:], AF.Identity, bias=b_sb[:, 0:1], scale=1.0
                )
                # corner turn: 4 x [32,128] -> one [128, 4*32] psum, one copy
                ptc = petpsp.tile([128, 128], F32, tag="pet")
                for q in range(GRP):
                    nc.tensor.transpose(
                        ptc[:, q * NB : (q + 1) * NB],
                        blt[:, q * 128 : (q + 1) * 128],
                        identf[0:NB, 0:NB],
                    )
                if b0 >= 2 * NCH:
                    # FMA chunks accumulate logits in f32
                    nc.vector.tensor_copy(
                        ZF[:, b0 - 2 * NCH : b0 - 2 * NCH + GRP, :], ptc[:]
                    )
                else:
                    nc.vector.tensor_copy(Z[:, b0 : b0 + GRP, :], ptc[:])
                emit()
                emit()
                while todo:
                    todo.pop(0)()

                if g == 11:
                    store_chunk(0, NCH)      # A finished during g11
            store_chunk(NCH, 2 * NCH)        # B finished during g15

            # ---------------- tail: C resumes at 16, D runs 0-31 ------
            ci, di = NCH, 0
            while ci < NB or di < NB:
                if di < NB:
                    fma_bin(di, NCH, rtD)
                    di += 1
                if ci < NB and (di % 2 == 0 or di >= NB):
                    fma_bin(ci, 0, rtC)
                    ci += 1
            store_chunk(2 * NCH, 3 * NCH)
            store_chunk(3 * NCH, B)

    orig = nc.to_json_bytes
    nc.to_json_bytes = lambda: _split_multiwait_json(orig())
    return nc


_PROG = None


def _get_prog():
    global _PROG
    if _PROG is None:
        _PROG = build_program()
    return _PROG


def _host_weights(W, b):
    """Host-side prep of the tiny weight tensors."""
    W = np.asarray(W, dtype=np.float32)
    wtr = np.zeros((128, 9, NB), dtype=ml_dtypes.bfloat16)
    for k in range(8):
        wtr[:, k, :] = W[:, k * 128 : (k + 1) * 128].T.astype(ml_dtypes.bfloat16)
    wtr[0:64, 8, :] = W[:, D : D + C].T.astype(ml_dtypes.bfloat16)
    wbin = W[:, D + C : DIN]  # [32, 32]
    vr = np.zeros((NB, NB), dtype=np.float32)
    for i in range(NB):
        vr[i, :i] = wbin[i, :i]
        vr[i, i] = 1.0
    vrows = np.broadcast_to(
        vr.astype(ml_dtypes.bfloat16)[None], (128, NB, NB)
    ).copy()
    bt = np.ascontiguousarray(
        np.tile(np.asarray(b, dtype=np.float32)[:, None], (1, 128))
    )
    tsub = np.zeros((128, NB), dtype=np.float32)
    for i in range(NB - 1):
        tsub[:, i] = wbin[i + 1, i]
    vcol = np.zeros((NB, NB), dtype=np.float32)
    for j in range(NB):
        vcol[j, j + 2 :] = wbin[j + 2 :, j]
    vcolf = np.broadcast_to(vcol[None], (128, NB, NB)).copy()
    return wtr, vrows, bt, tsub, vcolf


def kernel(features, word_class_features, W, b, trace=False, tmpdir=None):
    features = np.ascontiguousarray(features, dtype=np.float32)
    word_class_features = np.ascontiguousarray(word_class_features, dtype=np.float32)
    wtr, vrows, bf, tsub, vcolf = _host_weights(W, b)

    nc = _get_prog()
    in_maps = []
    for c in range(NCORES):
        sl = slice(c * NW, (c + 1) * NW)
        in_maps.append(
            {
                "feat": np.ascontiguousarray(features[:, sl, :]),
                "wc": np.ascontiguousarray(word_class_features[:, sl, :]),
                "wtr": wtr,
                "vrows": vrows,
                "b": bf,
                "ident": np.eye(128, dtype=np.float32),
                "tsub": tsub,
                "vcolf": vcolf,
            }
        )
    res = run_bass_kernel_spmd(
        nc, in_maps, core_ids=list(range(NCORES)), trace=trace, tmpdir=tmpdir
    )
    # per-core out is word-major [NW, B, NB]; transpose to [B, NW, NB]
    outp = np.concatenate(
        [
            res.results[c]["out"].astype(np.float32).transpose(1, 0, 2)
            for c in range(NCORES)
        ],
        axis=1,
    )
    kernel._last_result = res
    return outp
